# revision 1
# baseline (speedup 1.0000x reference)
"""Trainium2 Bass kernel for nn_Merge_MixtralSparseMoeBlock_14559939134022.

Math (see reference): all E experts alias one shared module, and the top-k
routing weights are renormalized to sum to 1 before being summed again, so
out = expert(x) * 1.0 exactly.  Only router_logits = x @ gate_w.T needs the
gate.  expert(x) = (silu(x@w1.T + (x@v1.T)@u1.T) * (x@w3.T + (x@v3.T)@u3.T))
projected back via w2/v2/u2.

Strategy: shard the 8192 tokens across 8 NeuronCores (1024 each), replicate
weights.  Host-side prep (untimed): transpose + bf16-cast all weights, pad the
low-rank dim R 398->512, concatenate [w1.T;u1.T], [w3.T;u3.T], [w2.T;u2.T] so
each projection is a single PSUM accumulation chain, and pre-tile every tensor
into its exact SBUF layout so all device DMAs are identity copies.

Per core, per 512-token pass:
  phase A: A1.T = v1 @ x.T, A3.T = v3 @ x.T, logits.T = gate_w @ x.T
  GEMM1:   for each of 56 I-blocks: gate.T/up.T (20 k-subtiles each),
           h.T = silu(gate.T)*up.T -> bf16
  BT:      B.T = v2p @ h.T (4 blocks of Rp)
  GEMM2:   out.T = [w2.T;u2p.T].T-contraction over [h.T;B.T] (60 k-subtiles)
"""

import numpy as np
import ml_dtypes
from einops import rearrange

import concourse.bass as bass
import concourse.mybir as mybir
import concourse.tile as tile
from concourse import bacc
from concourse.bass_utils import run_bass_kernel_spmd

BF16 = mybir.dt.bfloat16
F32 = mybir.dt.float32

B, S, H, I, E, R = 4, 2048, 2048, 7168, 8, 398
N = B * S                  # 8192 tokens
NCORES = 8
NTOK = N // NCORES         # 1024 tokens per core
NPASS = 2
T = NTOK // NPASS          # 512 tokens per pass
RP = 512                   # padded low-rank dim (4 k-subtiles)
P = 128

KS_X = H // P              # 16  x.T k-subtiles
KS_R = RP // P             # 4   low-rank k-subtiles
KS_G = KS_X + KS_R         # 20  [w1.T;u1.T] contraction subtiles
IB = I // P                # 56  I blocks
HB = H // P                # 16  H blocks
KS_O = IB + KS_R           # 60  [w2.T;u2p.T] contraction subtiles
MB_V = (RP + RP + P) // P  # 9   phase-A lhsT blocks (v1 | v3 | gate_w pad)
V2C = 4                    # v2 chunks per Rp block (56 = 4*14 subtiles)
V2K = IB // V2C            # 14


def _build_nc():
    nc = bacc.Bacc("TRN2", target_bir_lowering=False)

    x_in = nc.dram_tensor("x_in", [NPASS, P, KS_X * T], BF16, kind="ExternalInput")
    vj_in = nc.dram_tensor("vj_in", [MB_V, P, KS_X * P], BF16, kind="ExternalInput")
    wg_in = nc.dram_tensor("wg_in", [IB, P, KS_G * P], BF16, kind="ExternalInput")
    wu_in = nc.dram_tensor("wu_in", [IB, P, KS_G * P], BF16, kind="ExternalInput")
    wo_in = nc.dram_tensor("wo_in", [HB, P, KS_O * P], BF16, kind="ExternalInput")
    v2_in = nc.dram_tensor("v2_in", [KS_R, V2C, P, V2K * P], BF16, kind="ExternalInput")
    out_t = nc.dram_tensor("out_t", [HB, P, NTOK], F32, kind="ExternalOutput")
    logits_t = nc.dram_tensor("logits_t", [E, NTOK], F32, kind="ExternalOutput")

    with tile.TileContext(nc) as tc:
        with (
            tc.tile_pool(name="xa_pool", bufs=1) as xa_pool,
            tc.tile_pool(name="hb_pool", bufs=1) as hb_pool,
            tc.tile_pool(name="vt_pool", bufs=2) as vt_pool,
            tc.tile_pool(name="wg_pool", bufs=3) as wg_pool,
            tc.tile_pool(name="wo_pool", bufs=2) as wo_pool,
            tc.tile_pool(name="v2_pool", bufs=3) as v2_pool,
            tc.tile_pool(name="ev_pool", bufs=3) as ev_pool,
            tc.tile_pool(name="psum", bufs=6, space="PSUM") as psum_pool,
        ):
            for p in range(NPASS):
                # xa holds [x.T (ks 0..15) | A1.T (16..19) | A3.T (20..23)]
                xa = xa_pool.tile([P, KS_X + 2 * KS_R, T], BF16, tag="xa")
                nc.sync.dma_start(
                    xa[:, 0:KS_X, :],
                    x_in[p].rearrange("q (ks t) -> q ks t", t=T),
                )

                # ---- phase A: low-rank pre-projections + router logits ----
                for mb in range(MB_V):
                    vt = vt_pool.tile([P, KS_X * P], BF16, tag="vt")
                    nc.sync.dma_start(vt, vj_in[mb])
                    ps_a = psum_pool.tile([P, T], F32, tag="mm")
                    for ks in range(KS_X):
                        nc.tensor.matmul(
                            ps_a,
                            vt[:, ks * P : (ks + 1) * P],
                            xa[:, ks, :],
                            start=(ks == 0),
                            stop=(ks == KS_X - 1),
                        )
                    if mb < 2 * KS_R:
                        # A1.T rows (mb 0..3) and A3.T rows (mb 4..7), bf16
                        nc.vector.tensor_copy(out=xa[:, KS_X + mb, :], in_=ps_a)
                    else:
                        lg = ev_pool.tile([P, T], F32, tag="lg")
                        nc.scalar.copy(out=lg[:E, :], in_=ps_a[:E, :])
                        nc.sync.dma_start(
                            logits_t[:, p * T : (p + 1) * T], lg[:E, :]
                        )

                # ---- GEMM1: h.T = silu(gate.T) * up.T, streamed per I block ----
                hbt = hb_pool.tile([P, KS_O, T], BF16, tag="hbt")
                for ib in range(IB):
                    wg = wg_pool.tile([P, KS_G * P], BF16, tag="wg")
                    nc.sync.dma_start(wg, wg_in[ib])
                    wu = wg_pool.tile([P, KS_G * P], BF16, tag="wu")
                    nc.sync.dma_start(wu, wu_in[ib])

                    ps_g = psum_pool.tile([P, T], F32, tag="mm")
                    for j in range(KS_G):
                        # gate contraction reads xa ks j (x.T then A1.T)
                        nc.tensor.matmul(
                            ps_g,
                            wg[:, j * P : (j + 1) * P],
                            xa[:, j, :],
                            start=(j == 0),
                            stop=(j == KS_G - 1),
                        )
                    ps_u = psum_pool.tile([P, T], F32, tag="mm")
                    for j in range(KS_G):
                        ks = j if j < KS_X else j + KS_R  # x.T then A3.T
                        nc.tensor.matmul(
                            ps_u,
                            wu[:, j * P : (j + 1) * P],
                            xa[:, ks, :],
                            start=(j == 0),
                            stop=(j == KS_G - 1),
                        )
                    st = ev_pool.tile([P, T], F32, tag="st")
                    nc.scalar.activation(
                        st, ps_g, mybir.ActivationFunctionType.Silu
                    )
                    nc.vector.tensor_mul(out=hbt[:, ib, :], in0=st, in1=ps_u)

                # ---- BT: B.T = v2p @ h.T ----
                for m in range(KS_R):
                    ps_b = psum_pool.tile([P, T], F32, tag="mm")
                    for c in range(V2C):
                        v2 = v2_pool.tile([P, V2K * P], BF16, tag="v2")
                        nc.sync.dma_start(v2, v2_in[m, c])
                        for ik in range(V2K):
                            iks = c * V2K + ik
                            nc.tensor.matmul(
                                ps_b,
                                v2[:, ik * P : (ik + 1) * P],
                                hbt[:, iks, :],
                                start=(iks == 0),
                                stop=(iks == IB - 1),
                            )
                    nc.vector.tensor_copy(out=hbt[:, IB + m, :], in_=ps_b)

                # ---- GEMM2: out.T over [h.T; B.T] ----
                for hb in range(HB):
                    wo = wo_pool.tile([P, KS_O * P], BF16, tag="wo")
                    nc.sync.dma_start(wo, wo_in[hb])
                    ps_o = psum_pool.tile([P, T], F32, tag="mm")
                    for ks in range(KS_O):
                        nc.tensor.matmul(
                            ps_o,
                            wo[:, ks * P : (ks + 1) * P],
                            hbt[:, ks, :],
                            start=(ks == 0),
                            stop=(ks == KS_O - 1),
                        )
                    ot = ev_pool.tile([P, T], F32, tag="ot")
                    nc.scalar.copy(out=ot, in_=ps_o)
                    nc.sync.dma_start(out_t[hb, :, p * T : (p + 1) * T], ot)

    nc.compile()
    return nc


_NC_CACHE = None


def _get_nc():
    global _NC_CACHE
    if _NC_CACHE is None:
        _NC_CACHE = _build_nc()
    return _NC_CACHE


def _bf16(a):
    return np.ascontiguousarray(a).astype(ml_dtypes.bfloat16)


def _prep_inputs(x, gate_w, w1, w2, w3, u1, v1, u2, v2, u3, v3):
    f32 = np.float32
    x = np.asarray(x, f32).reshape(N, H)
    padc = lambda a: np.pad(np.asarray(a, f32), ((0, 0), (0, RP - R)))  # [*, R]->[*, RP]
    padr = lambda a: np.pad(np.asarray(a, f32), ((0, RP - R), (0, 0)))  # [R, *]->[RP, *]

    Wg = np.concatenate([np.asarray(w1, f32).T, padc(u1).T], axis=0)  # [H+RP, I]
    Wu = np.concatenate([np.asarray(w3, f32).T, padc(u3).T], axis=0)  # [H+RP, I]
    Wo = np.concatenate([np.asarray(w2, f32).T, padc(u2).T], axis=0)  # [I+RP, H]
    Vj = np.zeros((H, 2 * RP + P), f32)
    Vj[:, 0:R] = np.asarray(v1, f32).T
    Vj[:, RP : RP + R] = np.asarray(v3, f32).T
    Vj[:, 2 * RP : 2 * RP + E] = np.asarray(gate_w, f32).T
    V2 = padr(np.asarray(v2, f32)).T  # [I, RP]

    wg_b = rearrange(_bf16(Wg), "(ks q) (ib i) -> ib q (ks i)", q=P, i=P)
    wu_b = rearrange(_bf16(Wu), "(ks q) (ib i) -> ib q (ks i)", q=P, i=P)
    wo_b = rearrange(_bf16(Wo), "(ks q) (hb h) -> hb q (ks h)", q=P, h=P)
    vj_b = rearrange(_bf16(Vj), "(ks q) (mb m) -> mb q (ks m)", q=P, m=P)
    v2_b = rearrange(
        _bf16(V2), "(c ik q) (m r) -> m c q (ik r)", q=P, ik=V2K, r=P
    )

    shared = {
        "vj_in": np.ascontiguousarray(vj_b),
        "wg_in": np.ascontiguousarray(wg_b),
        "wu_in": np.ascontiguousarray(wu_b),
        "wo_in": np.ascontiguousarray(wo_b),
        "v2_in": np.ascontiguousarray(v2_b),
    }
    in_maps = []
    for c in range(NCORES):
        xc = _bf16(x[c * NTOK : (c + 1) * NTOK].T)  # [H, NTOK]
        xb = rearrange(xc, "(ks q) (p t) -> p q (ks t)", q=P, t=T)
        in_maps.append({"x_in": np.ascontiguousarray(xb), **shared})
    return in_maps


def _gather_outputs(results):
    out = np.empty((N, H), np.float32)
    logits = np.empty((N, E), np.float32)
    for c in range(NCORES):
        ot = np.asarray(results[c]["out_t"])  # [HB, P, NTOK]
        out[c * NTOK : (c + 1) * NTOK] = ot.reshape(H, NTOK).T
        logits[c * NTOK : (c + 1) * NTOK] = np.asarray(results[c]["logits_t"]).T
    return out.reshape(B, S, H), logits


def run(trace=False, **inputs):
    nc = _get_nc()
    in_maps = _prep_inputs(**inputs)
    res = run_bass_kernel_spmd(nc, in_maps, list(range(NCORES)), trace=trace)
    out, logits = _gather_outputs(res.results)
    return (out, logits), res


def kernel(**inputs):
    (out, logits), _ = run(trace=False, **inputs)
    return out, logits


# revision 4
# speedup vs baseline: 43.8549x; 43.8549x over previous
"""Trainium2 Bass kernel for nn_Merge_MixtralSparseMoeBlock_14559939134022.

Math (see reference): all E experts alias one shared module, and the top-k
routing weights are renormalized to sum to 1 before being summed again, so
out = expert(x) * 1.0 exactly.  Only router_logits = x @ gate_w.T needs the
gate.  expert(x) = (silu(x@w1.T + (x@v1.T)@u1.T) * (x@w3.T + (x@v3.T)@u3.T))
projected back via w2/v2/u2.

Strategy: shard the 8192 tokens across 8 NeuronCores (1024 each), replicate
weights.  Host-side prep (untimed): transpose + bf16-cast all weights, pad the
low-rank dim R 398->512, concatenate [w1.T;u1.T], [w3.T;u3.T], [w2.T;u2.T] so
each projection is a single PSUM accumulation chain, and pre-tile every tensor
into its exact SBUF layout so all device DMAs are identity copies.

Per core, per 512-token pass:
  phase A: A1.T = v1 @ x.T, A3.T = v3 @ x.T, logits.T = gate_w @ x.T
  GEMM1:   for each of 56 I-blocks: gate.T/up.T (20 k-subtiles each),
           h.T = silu(gate.T)*up.T -> bf16
  BT:      B.T = v2p @ h.T (4 blocks of Rp)
  GEMM2:   out.T = [w2.T;u2p.T].T-contraction over [h.T;B.T] (60 k-subtiles)
"""

import numpy as np
import ml_dtypes
from einops import rearrange

import concourse.bass as bass
import concourse.mybir as mybir
import concourse.tile as tile
from concourse import bacc
from concourse.bass_utils import run_bass_kernel_spmd

BF16 = mybir.dt.bfloat16
F32 = mybir.dt.float32

B, S, H, I, E, R = 4, 2048, 2048, 7168, 8, 398
N = B * S                  # 8192 tokens
NCORES = 8
NTOK = N // NCORES         # 1024 tokens per core
NPASS = 2
T = NTOK // NPASS          # 512 tokens per pass
RP = 512                   # padded low-rank dim (4 k-subtiles)
P = 128

KS_X = H // P              # 16  x.T k-subtiles
KS_R = RP // P             # 4   low-rank k-subtiles
KS_G = KS_X + KS_R         # 20  [w1.T;u1.T] contraction subtiles
IB = I // P                # 56  I blocks
HB = H // P                # 16  H blocks
KS_O = IB + KS_R           # 60  [w2.T;u2p.T] contraction subtiles
MB_V = (RP + RP + P) // P  # 9   phase-A lhsT blocks (v1 | v3 | gate_w pad)
V2C = 4                    # v2 chunks per Rp block (56 = 4*14 subtiles)
V2K = IB // V2C            # 14


def _build_nc(reps=1):
    nc = bacc.Bacc("TRN2", target_bir_lowering=False)

    x_in = nc.dram_tensor("x_in", [NPASS, P, KS_X * T], BF16, kind="ExternalInput")
    vj_in = nc.dram_tensor("vj_in", [MB_V, P, KS_X * P], BF16, kind="ExternalInput")
    wg_in = nc.dram_tensor("wg_in", [IB, P, KS_G * P], BF16, kind="ExternalInput")
    wu_in = nc.dram_tensor("wu_in", [IB, P, KS_G * P], BF16, kind="ExternalInput")
    wo_in = nc.dram_tensor("wo_in", [HB, P, KS_O * P], BF16, kind="ExternalInput")
    v2_in = nc.dram_tensor("v2_in", [KS_R, V2C, P, V2K * P], BF16, kind="ExternalInput")
    out_t = nc.dram_tensor("out_t", [HB, P, NTOK], F32, kind="ExternalOutput")
    logits_t = nc.dram_tensor("logits_t", [E, NTOK], F32, kind="ExternalOutput")

    with tile.TileContext(nc) as tc:
        with (
            tc.tile_pool(name="xa_pool", bufs=1) as xa_pool,
            tc.tile_pool(name="hb_pool", bufs=1) as hb_pool,
            tc.tile_pool(name="vt_pool", bufs=2) as vt_pool,
            tc.tile_pool(name="wg_pool", bufs=3) as wg_pool,
            tc.tile_pool(name="wo_pool", bufs=2) as wo_pool,
            tc.tile_pool(name="v2_pool", bufs=3) as v2_pool,
            tc.tile_pool(name="ev_pool", bufs=3) as ev_pool,
            tc.tile_pool(name="psum", bufs=6, space="PSUM") as psum_pool,
        ):
            for p in [pp % NPASS for pp in range(NPASS * reps)]:
                # xa holds [x.T (ks 0..15) | A1.T (16..19) | A3.T (20..23)]
                xa = xa_pool.tile([P, KS_X + 2 * KS_R, T], BF16, tag="xa")
                nc.sync.dma_start(
                    xa[:, 0:KS_X, :],
                    x_in[p].rearrange("q (ks t) -> q ks t", t=T),
                )

                # ---- phase A: low-rank pre-projections + router logits ----
                for mb in range(MB_V):
                    vt = vt_pool.tile([P, KS_X * P], BF16, tag="vt")
                    nc.sync.dma_start(vt, vj_in[mb])
                    ps_a = psum_pool.tile([P, T], F32, tag="mm")
                    for ks in range(KS_X):
                        nc.tensor.matmul(
                            ps_a,
                            vt[:, ks * P : (ks + 1) * P],
                            xa[:, ks, :],
                            start=(ks == 0),
                            stop=(ks == KS_X - 1),
                        )
                    if mb < 2 * KS_R:
                        # A1.T rows (mb 0..3) and A3.T rows (mb 4..7), bf16
                        nc.vector.tensor_copy(out=xa[:, KS_X + mb, :], in_=ps_a)
                    else:
                        lg = ev_pool.tile([P, T], F32, tag="lg")
                        nc.scalar.copy(out=lg[:E, :], in_=ps_a[:E, :])
                        nc.sync.dma_start(
                            logits_t[:, p * T : (p + 1) * T], lg[:E, :]
                        )

                # ---- GEMM1: h.T = silu(gate.T) * up.T, streamed per I block ----
                hbt = hb_pool.tile([P, KS_O, T], BF16, tag="hbt")
                for ib in range(IB):
                    wg = wg_pool.tile([P, KS_G * P], BF16, tag="wg")
                    nc.sync.dma_start(wg, wg_in[ib])
                    wu = wg_pool.tile([P, KS_G * P], BF16, tag="wu")
                    nc.sync.dma_start(wu, wu_in[ib])

                    ps_g = psum_pool.tile([P, T], F32, tag="mm")
                    for j in range(KS_G):
                        # gate contraction reads xa ks j (x.T then A1.T)
                        nc.tensor.matmul(
                            ps_g,
                            wg[:, j * P : (j + 1) * P],
                            xa[:, j, :],
                            start=(j == 0),
                            stop=(j == KS_G - 1),
                        )
                    ps_u = psum_pool.tile([P, T], F32, tag="mm")
                    for j in range(KS_G):
                        ks = j if j < KS_X else j + KS_R  # x.T then A3.T
                        nc.tensor.matmul(
                            ps_u,
                            wu[:, j * P : (j + 1) * P],
                            xa[:, ks, :],
                            start=(j == 0),
                            stop=(j == KS_G - 1),
                        )
                    st = ev_pool.tile([P, T], F32, tag="st")
                    nc.scalar.activation(
                        st, ps_g, mybir.ActivationFunctionType.Silu
                    )
                    nc.vector.tensor_mul(out=hbt[:, ib, :], in0=st, in1=ps_u)

                # ---- BT: B.T = v2p @ h.T ----
                for m in range(KS_R):
                    ps_b = psum_pool.tile([P, T], F32, tag="mm")
                    for c in range(V2C):
                        v2 = v2_pool.tile([P, V2K * P], BF16, tag="v2")
                        nc.sync.dma_start(v2, v2_in[m, c])
                        for ik in range(V2K):
                            iks = c * V2K + ik
                            nc.tensor.matmul(
                                ps_b,
                                v2[:, ik * P : (ik + 1) * P],
                                hbt[:, iks, :],
                                start=(iks == 0),
                                stop=(iks == IB - 1),
                            )
                    nc.vector.tensor_copy(out=hbt[:, IB + m, :], in_=ps_b)

                # ---- GEMM2: out.T over [h.T; B.T] ----
                for hb in range(HB):
                    wo = wo_pool.tile([P, KS_O * P], BF16, tag="wo")
                    nc.sync.dma_start(wo, wo_in[hb])
                    ps_o = psum_pool.tile([P, T], F32, tag="mm")
                    for ks in range(KS_O):
                        nc.tensor.matmul(
                            ps_o,
                            wo[:, ks * P : (ks + 1) * P],
                            hbt[:, ks, :],
                            start=(ks == 0),
                            stop=(ks == KS_O - 1),
                        )
                    ot = ev_pool.tile([P, T], F32, tag="ot")
                    nc.scalar.copy(out=ot, in_=ps_o)
                    nc.sync.dma_start(out_t[hb, :, p * T : (p + 1) * T], ot)

    nc.compile()
    return nc


_NC_CACHE = {}


def _get_nc(reps=1):
    if reps not in _NC_CACHE:
        _NC_CACHE[reps] = _build_nc(reps)
    return _NC_CACHE[reps]


def _bf16(a):
    return np.ascontiguousarray(a).astype(ml_dtypes.bfloat16)


def _prep_inputs(x, gate_w, w1, w2, w3, u1, v1, u2, v2, u3, v3):
    f32 = np.float32
    x = np.asarray(x, f32).reshape(N, H)
    padc = lambda a: np.pad(np.asarray(a, f32), ((0, 0), (0, RP - R)))  # [*, R]->[*, RP]
    padr = lambda a: np.pad(np.asarray(a, f32), ((0, RP - R), (0, 0)))  # [R, *]->[RP, *]

    Wg = np.concatenate([np.asarray(w1, f32).T, padc(u1).T], axis=0)  # [H+RP, I]
    Wu = np.concatenate([np.asarray(w3, f32).T, padc(u3).T], axis=0)  # [H+RP, I]
    Wo = np.concatenate([np.asarray(w2, f32).T, padc(u2).T], axis=0)  # [I+RP, H]
    Vj = np.zeros((H, 2 * RP + P), f32)
    Vj[:, 0:R] = np.asarray(v1, f32).T
    Vj[:, RP : RP + R] = np.asarray(v3, f32).T
    Vj[:, 2 * RP : 2 * RP + E] = np.asarray(gate_w, f32).T
    V2 = padr(np.asarray(v2, f32)).T  # [I, RP]

    wg_b = rearrange(_bf16(Wg), "(ks q) (ib i) -> ib q (ks i)", q=P, i=P)
    wu_b = rearrange(_bf16(Wu), "(ks q) (ib i) -> ib q (ks i)", q=P, i=P)
    wo_b = rearrange(_bf16(Wo), "(ks q) (hb h) -> hb q (ks h)", q=P, h=P)
    vj_b = rearrange(_bf16(Vj), "(ks q) (mb m) -> mb q (ks m)", q=P, m=P)
    v2_b = rearrange(
        _bf16(V2), "(c ik q) (m r) -> m c q (ik r)", q=P, ik=V2K, r=P
    )

    shared = {
        "vj_in": np.ascontiguousarray(vj_b),
        "wg_in": np.ascontiguousarray(wg_b),
        "wu_in": np.ascontiguousarray(wu_b),
        "wo_in": np.ascontiguousarray(wo_b),
        "v2_in": np.ascontiguousarray(v2_b),
    }
    in_maps = []
    for c in range(NCORES):
        xc = _bf16(x[c * NTOK : (c + 1) * NTOK].T)  # [H, NTOK]
        xb = rearrange(xc, "(ks q) (p t) -> p q (ks t)", q=P, t=T)
        in_maps.append({"x_in": np.ascontiguousarray(xb), **shared})
    return in_maps


def _gather_outputs(results):
    out = np.empty((N, H), np.float32)
    logits = np.empty((N, E), np.float32)
    for c in range(NCORES):
        ot = np.asarray(results[c]["out_t"])  # [HB, P, NTOK]
        out[c * NTOK : (c + 1) * NTOK] = ot.reshape(H, NTOK).T
        logits[c * NTOK : (c + 1) * NTOK] = np.asarray(results[c]["logits_t"]).T
    return out.reshape(B, S, H), logits


def run(trace=False, **inputs):
    nc = _get_nc()
    in_maps = _prep_inputs(**inputs)
    res = run_bass_kernel_spmd(nc, in_maps, list(range(NCORES)), trace=trace)
    out, logits = _gather_outputs(res.results)
    return (out, logits), res


def kernel(**inputs):
    (out, logits), _ = run(trace=False, **inputs)
    return out, logits
